# revision 14
# baseline (speedup 1.0000x reference)
"""RVQ (4-stage residual vector quantization) Trainium2 Bass kernel.

Problem (hardcoded): encoded_x [16, 256, 1024, 4] f32, codebooks [4, 512, 256] f32.
reference: x_in = transpose(encoded_x, (0,3,2,1)).reshape(-1, 256)  (N=65536 tokens)
4 stages of: d = ||r||^2 - 2 r.c + ||c||^2 ; idx = argmin_k d ; quant = c[idx];
out += quant ; r -= quant.  Outputs: (quantized [N,256] f32, indices [N,4] i32, loss [4] f32).

Sharding: batch dim (16) split across 8 cores (2 each; 8192 tokens/core).

Numerics: argmin needs fp32-class scores (min top-2 gap ~5e-5 vs typical score ~40).
fp32 PE matmul is 4 cyc/row; instead scores are computed with an exact fp16
double-double split at 1 cyc/row:
    r = rh + rl (fp16 pairs), c = ch + cl (fp16 pairs, host-precomputed)
    score = rh.ch + rh.cl + rl.ch + bias         (error ~2^-22, same flips as fp32)
    bias  = -0.5||c||^2 injected as a K=3 fp16 matmul (ones x 3-way fp16 split)

Device algorithm per 128-token tile (t = tokens on partitions):
  scores PSUM [128t, 512k] = 7 fp16 matmuls (1 bias + 6 split chunks)
  m = max_k scores                (DVE max8, PSUM src)
  idx = sum_k (scores==m)*iota_k  (DVE scalar_tensor_tensor with accum_out)
  gather codebook rows via indirect DMA (accumulating into the output tile across stages)
  residual: residT = xT - transpose(out_acc) (PE transpose + DVE subtract), then
  rh (ACT cast) / rl (GPSIMD subtract, fp16 out)
  loss via d_min = ||r||^2 - 2m => needs only sum(x^2) and per-token maxes (host-combined)
"""

import numpy as np
from contextlib import ExitStack

import concourse.bass as bass
import concourse.bacc as bacc
import concourse.tile as tile
from concourse import mybir, bass_utils
from concourse.bass import IndirectOffsetOnAxis
from concourse.masks import make_identity

B, C, H, W = 16, 256, 1024, 4
NCB, K, D = 4, 512, 256
NCORES = 8
BPC = B // NCORES              # batches per core (2)
HBLK = 32                      # h rows per tile -> 32*4 = 128 tokens
NHB = H // HBLK                # 32 h-blocks
NTILES = BPC * NHB             # 64 tiles/core
TOKPC = BPC * H * W            # 8192 tokens/core

F32 = mybir.dt.float32
F16 = mybir.dt.float16
U32 = mybir.dt.uint32
I32 = mybir.dt.int32
AX = mybir.AluOpType
AF = mybir.ActivationFunctionType


def build_kernel(ntiles=NTILES):
    nc = bacc.Bacc("TRN2", target_bir_lowering=False, debug=False, num_devices=NCORES)
    x = nc.dram_tensor("x", [BPC, C, H, W], F32, kind="ExternalInput")
    cbh = nc.dram_tensor("cbh", [NCB, D, K], F16, kind="ExternalInput")
    cbl = nc.dram_tensor("cbl", [NCB, D, K], F16, kind="ExternalInput")
    cbs = [nc.dram_tensor(f"cb{q}", [K, D], F32, kind="ExternalInput") for q in range(NCB)]
    bias3 = nc.dram_tensor("bias3", [NCB, 3, K], F16, kind="ExternalInput")
    out_q = nc.dram_tensor("out_q", [TOKPC, D], F32, kind="ExternalOutput")
    out_i = nc.dram_tensor("out_i", [TOKPC, NCB], U32, kind="ExternalOutput")
    out_m = nc.dram_tensor("out_m", [NTILES, 128, 8 * NCB], F32, kind="ExternalOutput")
    out_xsq = nc.dram_tensor("out_xsq", [NTILES, 128, 2], F32, kind="ExternalOutput")

    # token tile (b, hb): tokens t_local = b*4096 + w*1024 + hb*32 + hh ; partition p = hh*4 + w
    xr = x.ap().rearrange("b c (hb hh) w -> b hb c (hh w)", hh=HBLK)
    oq = out_q.ap().rearrange("(b w hb hh) c -> b hb hh w c", w=W, hb=NHB, hh=HBLK)
    oi = out_i.ap().rearrange("(b w hb hh) c -> b hb hh w c", w=W, hb=NHB, hh=HBLK)

    with tile.TileContext(nc) as tc, ExitStack() as ctx:
        const = ctx.enter_context(tc.tile_pool(name="const", bufs=1))
        xt_pool = ctx.enter_context(tc.tile_pool(name="xt", bufs=3))
        resid_pool = ctx.enter_context(tc.tile_pool(name="resid", bufs=4))
        rh_pool = ctx.enter_context(tc.tile_pool(name="rh", bufs=6))
        rl_pool = ctx.enter_context(tc.tile_pool(name="rl", bufs=6))
        outq_pool = ctx.enter_context(tc.tile_pool(name="outq", bufs=3))
        m8_pool = ctx.enter_context(tc.tile_pool(name="m8", bufs=3))
        idx_pool = ctx.enter_context(tc.tile_pool(name="idx", bufs=3))
        small_pool = ctx.enter_context(tc.tile_pool(name="small", bufs=4))
        ps_scores = ctx.enter_context(tc.tile_pool(name="ps_s", bufs=2, space="PSUM"))
        ps_tp = ctx.enter_context(tc.tile_pool(name="ps_tp", bufs=2, space="PSUM"))
        ps_trash = ctx.enter_context(tc.tile_pool(name="ps_tr", bufs=1, space="PSUM"))

        # ---- static tiles ----
        identity = const.tile([128, 128], F32, tag="ident")
        make_identity(nc, identity[:])
        ones3 = const.tile([3, 128], F16, tag="ones3")
        nc.gpsimd.memset(ones3[:], 1.0)
        iota_i = const.tile([128, K], I32, tag="iota_i")
        nc.gpsimd.iota(iota_i[:], pattern=[[1, K]], base=0, channel_multiplier=0)
        iota_f = const.tile([128, K], F32, tag="iota_f")
        nc.vector.tensor_copy(iota_f[:], iota_i[:])

        cbh_sb, cbl_sb, bias_sb = [], [], []
        for q in range(NCB):
            hs, ls = [], []
            for j in range(2):
                th = const.tile([128, K], F16, tag=f"cbh{q}_{j}")
                nc.sync.dma_start(th[:], cbh.ap()[q, 128 * j:128 * (j + 1), :])
                hs.append(th)
                tl = const.tile([128, K], F16, tag=f"cbl{q}_{j}")
                nc.sync.dma_start(tl[:], cbl.ap()[q, 128 * j:128 * (j + 1), :])
                ls.append(tl)
            cbh_sb.append(hs)
            cbl_sb.append(ls)
            tb = const.tile([3, K], F16, tag=f"bias{q}")
            nc.sync.dma_start(tb[:], bias3.ap()[q])
            bias_sb.append(tb)

        # ---- main loop over token tiles ----
        for tix in range(ntiles):
            b, hb = divmod(tix, NHB)

            # load xT tile [128 c-part, 2 c-chunks x 128 tokens]
            xt = xt_pool.tile([128, 2 * 128], F32, tag="xt")
            for j in range(2):
                nc.sync.dma_start(xt[:, 128 * j:128 * (j + 1)],
                                  xr[b, hb][128 * j:128 * (j + 1), :])

            # per-token sum(x^2): ACT square, accum along free dim per d-lane
            xsq = small_pool.tile([128, 2], F32, tag="xsq")
            for j in range(2):
                tr = ps_trash.tile([128, K], F32, tag="trash")
                nc.scalar.activation(
                    tr[:, 0:128], xt[:, 128 * j:128 * (j + 1)],
                    AF.Square, accum_out=xsq[:, j:j + 1])

            m8 = m8_pool.tile([128, 8 * NCB], F32, tag="m8")
            idxs = idx_pool.tile([128, NCB], U32, tag="idxs")
            outq_t = outq_pool.tile([128, D], F32, tag="outq")

            resid = xt
            for q in range(NCB):
                # fp16 split of the residual: rh = fp16(r); rl = fp16(r - rh)
                rh = rh_pool.tile([128, 2 * 128], F16, tag="rh")
                nc.scalar.copy(rh[:], resid[:])
                rl = rl_pool.tile([128, 2 * 128], F16, tag="rl")
                nc.gpsimd.tensor_tensor(rl[:], resid[:], rh[:], op=AX.subtract)

                # scores = bias + rh.ch + rh.cl + rl.ch   (PSUM accumulate)
                ps = ps_scores.tile([128, K], F32, tag="scores")
                nc.tensor.matmul(ps[:], ones3[:], bias_sb[q][:],
                                 start=True, stop=False)
                for j in range(2):
                    sl = slice(128 * j, 128 * (j + 1))
                    nc.tensor.matmul(ps[:], rh[:, sl], cbh_sb[q][j][:],
                                     start=False, stop=False)
                    nc.tensor.matmul(ps[:], rh[:, sl], cbl_sb[q][j][:],
                                     start=False, stop=False)
                    nc.tensor.matmul(ps[:], rl[:, sl], cbh_sb[q][j][:],
                                     start=False, stop=(j == 1))

                # max + argmax
                nc.vector.max(m8[:, 8 * q:8 * q + 8], ps[:])
                mi = ps_trash.tile([128, K], F32, tag="trash")
                idxf = small_pool.tile([128, 1], F32, tag="idxf")
                nc.vector.scalar_tensor_tensor(
                    mi[:], ps[:], m8[:, 8 * q:8 * q + 1], iota_f[:],
                    op0=AX.is_equal, op1=AX.mult, accum_out=idxf[:])
                nc.vector.tensor_scalar_min(idxf[:], idxf[:], float(K - 1))
                nc.vector.tensor_copy(idxs[:, q:q + 1], idxf[:])

                # gather codebook rows; accumulate into output tile across stages
                nc.gpsimd.indirect_dma_start(
                    out=outq_t[:],
                    out_offset=None,
                    in_=cbs[q].ap()[:],
                    in_offset=IndirectOffsetOnAxis(ap=idxs[:, q:q + 1], axis=0),
                    compute_op=(AX.bypass if q == 0 else AX.add))

                if q < NCB - 1:
                    # residT = xT - transpose(out_acc)
                    tp = ps_tp.tile([128, 2 * 128], F32, tag="tp")
                    for j in range(2):
                        nc.tensor.transpose(
                            tp[:, 128 * j:128 * (j + 1)],
                            outq_t[:, 128 * j:128 * (j + 1)], identity[:])
                    resid_new = resid_pool.tile([128, 2 * 128], F32, tag="resid")
                    nc.vector.tensor_tensor(
                        resid_new[:], xt[:], tp[:], op=AX.subtract)
                    resid = resid_new

            # stores
            nc.sync.dma_start(oq[b, hb], outq_t[:])
            nc.sync.dma_start(oi[b, hb], idxs[:])
            nc.sync.dma_start(out_m.ap()[tix], m8[:])
            nc.sync.dma_start(out_xsq.ap()[tix], xsq[:])

    nc.compile()
    return nc


def _split16(a):
    """exact 2-way fp16 split: a ~= h + l (h,l fp16)"""
    h = a.astype(np.float16)
    l = (a.astype(np.float64) - h.astype(np.float64)).astype(np.float16)
    return h, l


def _prep_host(encoded_x, codebooks):
    x = np.ascontiguousarray(np.asarray(encoded_x), dtype=np.float32)
    cb = np.ascontiguousarray(np.asarray(codebooks), dtype=np.float32)
    cbt = np.ascontiguousarray(np.transpose(cb, (0, 2, 1)))       # [4, 256, 512]
    ch, cl = _split16(cbt)
    b64 = -0.5 * (cb.astype(np.float64) ** 2).sum(-1)             # [4, 512]
    b_h = b64.astype(np.float16)
    r1 = b64 - b_h.astype(np.float64)
    b_m = r1.astype(np.float16)
    r2 = r1 - b_m.astype(np.float64)
    b_l = r2.astype(np.float16)
    bias3 = np.ascontiguousarray(np.stack([b_h, b_m, b_l], axis=1))  # [4, 3, 512]
    in_maps = []
    for i in range(NCORES):
        m = {"x": np.ascontiguousarray(x[i * BPC:(i + 1) * BPC]),
             "cbh": ch, "cbl": cl, "bias3": bias3}
        for q in range(NCB):
            m[f"cb{q}"] = cb[q]
        in_maps.append(m)
    return in_maps


def _assemble(results):
    qo = np.concatenate([results[i]["out_q"] for i in range(NCORES)], axis=0)
    idx = np.concatenate([results[i]["out_i"] for i in range(NCORES)],
                         axis=0).astype(np.int32)
    sum_xsq = sum(results[i]["out_xsq"].sum(dtype=np.float64) for i in range(NCORES))
    summax = [sum(results[i]["out_m"][:, :, 8 * q].sum(dtype=np.float64)
                  for i in range(NCORES)) for q in range(NCB)]
    n_el = float(B * H * W) * D
    losses, csum = [], 0.0
    for q in range(NCB):
        csum += summax[q]
        losses.append((sum_xsq - 2.0 * csum) / n_el)
    loss = np.array(losses, dtype=np.float32)
    return qo, idx, loss


_CACHED_NC = None


def kernel(encoded_x, codebooks):
    global _CACHED_NC
    if _CACHED_NC is None:
        _CACHED_NC = build_kernel()
    in_maps = _prep_host(encoded_x, codebooks)
    res = bass_utils.run_bass_kernel_spmd(_CACHED_NC, in_maps,
                                          core_ids=list(range(NCORES)))
    return _assemble(res.results)


# revision 17
# speedup vs baseline: 1.1539x; 1.1539x over previous
"""RVQ (4-stage residual vector quantization) Trainium2 Bass kernel.

Problem (hardcoded): encoded_x [16, 256, 1024, 4] f32, codebooks [4, 512, 256] f32.
reference: x_in = transpose(encoded_x, (0,3,2,1)).reshape(-1, 256)  (N=65536 tokens)
4 stages of: d = ||r||^2 - 2 r.c + ||c||^2 ; idx = argmin_k d ; quant = c[idx];
out += quant ; r -= quant.  Outputs: (quantized [N,256] f32, indices [N,4] i32, loss [4] f32).

Sharding: batch dim (16) split across 8 cores (2 each; 8192 tokens/core).

Numerics: argmin needs fp32-class scores (min top-2 gap ~5e-5 vs typical score ~40).
fp32 PE matmul is 4 cyc/row; instead scores are computed with an exact fp16
double-double split at 1 cyc/row:
    r = rh + rl (fp16 pairs), c = ch + cl (fp16 pairs, host-precomputed)
    score = rh.ch + rh.cl + rl.ch + bias         (error ~2^-22, same flips as fp32)
    bias  = -0.5||c||^2 injected as a K=3 fp16 matmul (ones x 3-way fp16 split)

Device algorithm per 128-token tile (t = tokens on partitions):
  scores PSUM [128t, 512k] = 7 fp16 matmuls (1 bias + 6 split chunks)
  m = max_k scores                (DVE max8, PSUM src)
  idx = sum_k (scores==m)*iota_k  (DVE scalar_tensor_tensor with accum_out)
  gather codebook rows via indirect DMA (accumulating into the output tile across stages)
  residual: residT = xT - transpose(out_acc) (PE transpose + DVE subtract), then
  rh (ACT cast) / rl (GPSIMD subtract, fp16 out)
  loss via d_min = ||r||^2 - 2m => needs only sum(x^2) and per-token maxes (host-combined)
"""

import numpy as np
from contextlib import ExitStack

import concourse.bass as bass
import concourse.bacc as bacc
import concourse.tile as tile
from concourse import mybir, bass_utils
from concourse.bass import IndirectOffsetOnAxis
from concourse.masks import make_identity

B, C, H, W = 16, 256, 1024, 4
NCB, K, D = 4, 512, 256
NCORES = 8
BPC = B // NCORES              # batches per core (2)
HBLK = 32                      # h rows per tile -> 32*4 = 128 tokens
NHB = H // HBLK                # 32 h-blocks
NTILES = BPC * NHB             # 64 tiles/core
TOKPC = BPC * H * W            # 8192 tokens/core

F32 = mybir.dt.float32
F16 = mybir.dt.float16
U32 = mybir.dt.uint32
I32 = mybir.dt.int32
AX = mybir.AluOpType
AF = mybir.ActivationFunctionType


def build_kernel(ntiles=NTILES):
    nc = bacc.Bacc("TRN2", target_bir_lowering=False, debug=False, num_devices=NCORES)
    x = nc.dram_tensor("x", [BPC, C, H, W], F32, kind="ExternalInput")
    cbh = nc.dram_tensor("cbh", [NCB, D, K], F16, kind="ExternalInput")
    cbl = nc.dram_tensor("cbl", [NCB, D, K], F16, kind="ExternalInput")
    cbs = [nc.dram_tensor(f"cb{q}", [K, D], F32, kind="ExternalInput") for q in range(NCB)]
    bias3 = nc.dram_tensor("bias3", [NCB, 3, K], F16, kind="ExternalInput")
    out_q = nc.dram_tensor("out_q", [TOKPC, D], F32, kind="ExternalOutput")
    out_i = nc.dram_tensor("out_i", [TOKPC, NCB], U32, kind="ExternalOutput")
    out_m = nc.dram_tensor("out_m", [NTILES, 128, 8 * NCB], F32, kind="ExternalOutput")
    out_xsq = nc.dram_tensor("out_xsq", [NTILES, 128, 2], F32, kind="ExternalOutput")

    # token tile (b, hb): tokens t_local = b*4096 + w*1024 + hb*32 + hh ; partition p = hh*4 + w
    xr = x.ap().rearrange("b c (hb hh) w -> b hb c (hh w)", hh=HBLK)
    oq = out_q.ap().rearrange("(b w hb hh) c -> b hb hh w c", w=W, hb=NHB, hh=HBLK)
    oi = out_i.ap().rearrange("(b w hb hh) c -> b hb hh w c", w=W, hb=NHB, hh=HBLK)

    with tile.TileContext(nc) as tc, ExitStack() as ctx:
        const = ctx.enter_context(tc.tile_pool(name="const", bufs=1))
        xt_pool = ctx.enter_context(tc.tile_pool(name="xt", bufs=4))
        resid_pool = ctx.enter_context(tc.tile_pool(name="resid", bufs=6))
        rh_pool = ctx.enter_context(tc.tile_pool(name="rh", bufs=8))
        rl_pool = ctx.enter_context(tc.tile_pool(name="rl", bufs=8))
        outq_pool = ctx.enter_context(tc.tile_pool(name="outq", bufs=4))
        m8_pool = ctx.enter_context(tc.tile_pool(name="m8", bufs=4))
        idx_pool = ctx.enter_context(tc.tile_pool(name="idx", bufs=4))
        small_pool = ctx.enter_context(tc.tile_pool(name="small", bufs=8))
        scratch_pool = ctx.enter_context(tc.tile_pool(name="scratch", bufs=2))
        ps_scores = ctx.enter_context(tc.tile_pool(name="ps_s", bufs=3, space="PSUM"))
        ps_tp = ctx.enter_context(tc.tile_pool(name="ps_tp", bufs=3, space="PSUM"))

        # ---- static tiles ----
        identity = const.tile([128, 128], F32, tag="ident")
        make_identity(nc, identity[:])
        ones3 = const.tile([3, 128], F16, tag="ones3")
        nc.gpsimd.memset(ones3[:], 1.0)
        iota_i = const.tile([128, K], I32, tag="iota_i")
        nc.gpsimd.iota(iota_i[:], pattern=[[1, K]], base=0, channel_multiplier=0)
        iota_f = const.tile([128, K], F32, tag="iota_f")
        nc.vector.tensor_copy(iota_f[:], iota_i[:])

        cbh_sb, cbl_sb, bias_sb = [], [], []
        for q in range(NCB):
            hs, ls = [], []
            for j in range(2):
                th = const.tile([128, K], F16, tag=f"cbh{q}_{j}")
                nc.sync.dma_start(th[:], cbh.ap()[q, 128 * j:128 * (j + 1), :])
                hs.append(th)
                tl = const.tile([128, K], F16, tag=f"cbl{q}_{j}")
                nc.sync.dma_start(tl[:], cbl.ap()[q, 128 * j:128 * (j + 1), :])
                ls.append(tl)
            cbh_sb.append(hs)
            cbl_sb.append(ls)
            tb = const.tile([3, K], F16, tag=f"bias{q}")
            nc.sync.dma_start(tb[:], bias3.ap()[q])
            bias_sb.append(tb)

        # ---- main loop over token tiles ----
        for tix in range(ntiles):
            b, hb = divmod(tix, NHB)

            # load xT tile [128 c-part, 2 c-chunks x 128 tokens]
            xt = xt_pool.tile([128, 2 * 128], F32, tag="xt")
            for j in range(2):
                nc.sync.dma_start(xt[:, 128 * j:128 * (j + 1)],
                                  xr[b, hb][128 * j:128 * (j + 1), :])

            # per-token sum(x^2): ACT square, accum along free dim per d-lane
            xsq = small_pool.tile([128, 2], F32, tag="xsq")
            for j in range(2):
                tr = scratch_pool.tile([128, 128], F32, tag="sqtrash")
                nc.scalar.activation(
                    tr[:], xt[:, 128 * j:128 * (j + 1)],
                    AF.Square, accum_out=xsq[:, j:j + 1])

            m8 = m8_pool.tile([128, 8 * NCB], F32, tag="m8")
            idxs = idx_pool.tile([128, NCB], U32, tag="idxs")
            outq_t = outq_pool.tile([128, D], F32, tag="outq")

            resid = xt
            for q in range(NCB):
                # fp16 split of the residual: rh = fp16(r); rl = fp16(r - rh)
                rh = rh_pool.tile([128, 2 * 128], F16, tag="rh")
                nc.scalar.copy(rh[:], resid[:])
                rl = rl_pool.tile([128, 2 * 128], F16, tag="rl")
                nc.gpsimd.tensor_tensor(rl[:], resid[:], rh[:], op=AX.subtract)

                # scores = bias + rh.ch + rh.cl + rl.ch   (PSUM accumulate)
                ps = ps_scores.tile([128, K], F32, tag="scores")
                nc.tensor.matmul(ps[:], ones3[:], bias_sb[q][:],
                                 start=True, stop=False)
                for j in range(2):
                    sl = slice(128 * j, 128 * (j + 1))
                    nc.tensor.matmul(ps[:], rh[:, sl], cbh_sb[q][j][:],
                                     start=False, stop=False)
                    nc.tensor.matmul(ps[:], rh[:, sl], cbl_sb[q][j][:],
                                     start=False, stop=False)
                    nc.tensor.matmul(ps[:], rl[:, sl], cbh_sb[q][j][:],
                                     start=False, stop=(j == 1))

                # max + argmax (STT overwrites the dead scores bank in place)
                nc.vector.max(m8[:, 8 * q:8 * q + 8], ps[:])
                idxf = small_pool.tile([128, 1], F32, tag="idxf")
                nc.vector.scalar_tensor_tensor(
                    ps[:], ps[:], m8[:, 8 * q:8 * q + 1], iota_f[:],
                    op0=AX.is_equal, op1=AX.mult, accum_out=idxf[:])
                nc.vector.tensor_scalar_min(idxf[:], idxf[:], float(K - 1))
                nc.vector.tensor_copy(idxs[:, q:q + 1], idxf[:])

                # gather codebook rows; accumulate into output tile across stages
                nc.gpsimd.indirect_dma_start(
                    out=outq_t[:],
                    out_offset=None,
                    in_=cbs[q].ap()[:],
                    in_offset=IndirectOffsetOnAxis(ap=idxs[:, q:q + 1], axis=0),
                    compute_op=(AX.bypass if q == 0 else AX.add))

                if q < NCB - 1:
                    # residT = xT - transpose(out_acc)
                    tp = ps_tp.tile([128, 2 * 128], F32, tag="tp")
                    for j in range(2):
                        nc.tensor.transpose(
                            tp[:, 128 * j:128 * (j + 1)],
                            outq_t[:, 128 * j:128 * (j + 1)], identity[:])
                    resid_new = resid_pool.tile([128, 2 * 128], F32, tag="resid")
                    nc.vector.tensor_tensor(
                        resid_new[:], xt[:], tp[:], op=AX.subtract)
                    resid = resid_new

            # stores
            nc.sync.dma_start(oq[b, hb], outq_t[:])
            nc.sync.dma_start(oi[b, hb], idxs[:])
            nc.sync.dma_start(out_m.ap()[tix], m8[:])
            nc.sync.dma_start(out_xsq.ap()[tix], xsq[:])

    nc.compile()
    return nc


def _split16(a):
    """exact 2-way fp16 split: a ~= h + l (h,l fp16)"""
    h = a.astype(np.float16)
    l = (a.astype(np.float64) - h.astype(np.float64)).astype(np.float16)
    return h, l


def _prep_host(encoded_x, codebooks):
    x = np.ascontiguousarray(np.asarray(encoded_x), dtype=np.float32)
    cb = np.ascontiguousarray(np.asarray(codebooks), dtype=np.float32)
    cbt = np.ascontiguousarray(np.transpose(cb, (0, 2, 1)))       # [4, 256, 512]
    ch, cl = _split16(cbt)
    b64 = -0.5 * (cb.astype(np.float64) ** 2).sum(-1)             # [4, 512]
    b_h = b64.astype(np.float16)
    r1 = b64 - b_h.astype(np.float64)
    b_m = r1.astype(np.float16)
    r2 = r1 - b_m.astype(np.float64)
    b_l = r2.astype(np.float16)
    bias3 = np.ascontiguousarray(np.stack([b_h, b_m, b_l], axis=1))  # [4, 3, 512]
    in_maps = []
    for i in range(NCORES):
        m = {"x": np.ascontiguousarray(x[i * BPC:(i + 1) * BPC]),
             "cbh": ch, "cbl": cl, "bias3": bias3}
        for q in range(NCB):
            m[f"cb{q}"] = cb[q]
        in_maps.append(m)
    return in_maps


def _assemble(results):
    qo = np.concatenate([results[i]["out_q"] for i in range(NCORES)], axis=0)
    idx = np.concatenate([results[i]["out_i"] for i in range(NCORES)],
                         axis=0).astype(np.int32)
    sum_xsq = sum(results[i]["out_xsq"].sum(dtype=np.float64) for i in range(NCORES))
    summax = [sum(results[i]["out_m"][:, :, 8 * q].sum(dtype=np.float64)
                  for i in range(NCORES)) for q in range(NCB)]
    n_el = float(B * H * W) * D
    losses, csum = [], 0.0
    for q in range(NCB):
        csum += summax[q]
        losses.append((sum_xsq - 2.0 * csum) / n_el)
    loss = np.array(losses, dtype=np.float32)
    return qo, idx, loss


_CACHED_NC = None


def kernel(encoded_x, codebooks):
    global _CACHED_NC
    if _CACHED_NC is None:
        _CACHED_NC = build_kernel()
    in_maps = _prep_host(encoded_x, codebooks)
    res = bass_utils.run_bass_kernel_spmd(_CACHED_NC, in_maps,
                                          core_ids=list(range(NCORES)))
    return _assemble(res.results)


# revision 20
# speedup vs baseline: 2.6169x; 2.2678x over previous
"""RVQ (4-stage residual vector quantization) Trainium2 Bass kernel.

Problem (hardcoded): encoded_x [16, 256, 1024, 4] f32, codebooks [4, 512, 256] f32.
reference: x_in = transpose(encoded_x, (0,3,2,1)).reshape(-1, 256)  (N=65536 tokens)
4 stages of: d = ||r||^2 - 2 r.c + ||c||^2 ; idx = argmin_k d ; quant = c[idx];
out += quant ; r -= quant.  Outputs: (quantized [N,256] f32, indices [N,4] i32, loss [4] f32).

Sharding: batch dim (16) split across 8 cores (2 each; 8192 tokens/core).

Numerics: argmin needs fp32-class scores (min top-2 gap ~5e-5 vs typical score ~40).
fp32 PE matmul is 4 cyc/row; instead scores are computed with an exact fp16
double-double split at 1 cyc/row:
    r = rh + rl (fp16 pairs), c = ch + cl (fp16 pairs, host-precomputed)
    score = rh.ch + rh.cl + rl.ch + bias         (error ~2^-22, same flips as fp32)
    bias  = -0.5||c||^2 injected as a K=3 fp16 matmul (ones x 3-way fp16 split)

Device algorithm per 128-token tile (t = tokens on partitions):
  scores PSUM [128t, 512k] = 7 fp16 matmuls (1 bias + 6 split chunks)
  m = max_k scores                (DVE max8, PSUM src)
  idx = sum_k (scores==m)*iota_k  (DVE scalar_tensor_tensor with accum_out)
  gather codebook rows via indirect DMA (accumulating into the output tile across stages)
  residual: residT = xT - transpose(out_acc) (PE transpose + DVE subtract), then
  rh (ACT cast) / rl (GPSIMD subtract, fp16 out)
  loss via d_min = ||r||^2 - 2m => needs only sum(x^2) and per-token maxes (host-combined)
"""

import numpy as np
from contextlib import ExitStack

import concourse.bass as bass
import concourse.bacc as bacc
import concourse.tile as tile
from concourse import mybir, bass_utils
from concourse.bass import IndirectOffsetOnAxis
from concourse.masks import make_identity

B, C, H, W = 16, 256, 1024, 4
NCB, K, D = 4, 512, 256
NCORES = 8
BPC = B // NCORES              # batches per core (2)
HBLK = 32                      # h rows per tile -> 32*4 = 128 tokens
NHB = H // HBLK                # 32 h-blocks
NTILES = BPC * NHB             # 64 tiles/core
TOKPC = BPC * H * W            # 8192 tokens/core

F32 = mybir.dt.float32
F16 = mybir.dt.float16
U32 = mybir.dt.uint32
I32 = mybir.dt.int32
AX = mybir.AluOpType
AF = mybir.ActivationFunctionType


def build_kernel(ntiles=NTILES):
    nc = bacc.Bacc("TRN2", target_bir_lowering=False, debug=False, num_devices=NCORES)
    x = nc.dram_tensor("x", [BPC, C, H, W], F32, kind="ExternalInput")
    cbh = nc.dram_tensor("cbh", [NCB, D, K], F16, kind="ExternalInput")
    cbl = nc.dram_tensor("cbl", [NCB, D, K], F16, kind="ExternalInput")
    cbs = [nc.dram_tensor(f"cb{q}", [K, D], F32, kind="ExternalInput") for q in range(NCB)]
    bias3 = nc.dram_tensor("bias3", [NCB, 3, K], F16, kind="ExternalInput")
    out_q = nc.dram_tensor("out_q", [TOKPC, D], F32, kind="ExternalOutput")
    out_i = nc.dram_tensor("out_i", [TOKPC, NCB], U32, kind="ExternalOutput")
    out_m = nc.dram_tensor("out_m", [NTILES, 128, 8 * NCB], F32, kind="ExternalOutput")
    out_xsq = nc.dram_tensor("out_xsq", [NTILES, 128, 2], F32, kind="ExternalOutput")

    # token tile (b, hb): tokens t_local = b*4096 + w*1024 + hb*32 + hh ; partition p = hh*4 + w
    xr = x.ap().rearrange("b c (hb hh) w -> b hb c (hh w)", hh=HBLK)
    oq = out_q.ap().rearrange("(b w hb hh) c -> b hb hh w c", w=W, hb=NHB, hh=HBLK)
    oi = out_i.ap().rearrange("(b w hb hh) c -> b hb hh w c", w=W, hb=NHB, hh=HBLK)

    with tile.TileContext(nc) as tc, ExitStack() as ctx:
        const = ctx.enter_context(tc.tile_pool(name="const", bufs=1))
        xt_pool = ctx.enter_context(tc.tile_pool(name="xt", bufs=9))
        resid_pool = ctx.enter_context(tc.tile_pool(name="resid", bufs=10))
        rh_pool = ctx.enter_context(tc.tile_pool(name="rh", bufs=9))
        rl_pool = ctx.enter_context(tc.tile_pool(name="rl", bufs=9))
        outq_pool = ctx.enter_context(tc.tile_pool(name="outq", bufs=9))
        m8_pool = ctx.enter_context(tc.tile_pool(name="m8", bufs=9))
        idx_pool = ctx.enter_context(tc.tile_pool(name="idx", bufs=9))
        small_pool = ctx.enter_context(tc.tile_pool(name="small", bufs=18))
        scratch_pool = ctx.enter_context(tc.tile_pool(name="scratch", bufs=4))
        ps_scores = ctx.enter_context(tc.tile_pool(name="ps_s", bufs=4, space="PSUM"))
        ps_tp = ctx.enter_context(tc.tile_pool(name="ps_tp", bufs=4, space="PSUM"))

        # ---- static tiles ----
        identity = const.tile([128, 128], F32, tag="ident")
        make_identity(nc, identity[:])
        ones3 = const.tile([3, 128], F16, tag="ones3")
        nc.gpsimd.memset(ones3[:], 1.0)
        iota_i = const.tile([128, K], I32, tag="iota_i")
        nc.gpsimd.iota(iota_i[:], pattern=[[1, K]], base=0, channel_multiplier=0)
        iota_f = const.tile([128, K], F32, tag="iota_f")
        nc.vector.tensor_copy(iota_f[:], iota_i[:])

        cbh_sb, cbl_sb, bias_sb = [], [], []
        for q in range(NCB):
            hs, ls = [], []
            for j in range(2):
                th = const.tile([128, K], F16, tag=f"cbh{q}_{j}")
                nc.sync.dma_start(th[:], cbh.ap()[q, 128 * j:128 * (j + 1), :])
                hs.append(th)
                tl = const.tile([128, K], F16, tag=f"cbl{q}_{j}")
                nc.sync.dma_start(tl[:], cbl.ap()[q, 128 * j:128 * (j + 1), :])
                ls.append(tl)
            cbh_sb.append(hs)
            cbl_sb.append(ls)
            tb = const.tile([3, K], F16, tag=f"bias{q}")
            nc.sync.dma_start(tb[:], bias3.ap()[q])
            bias_sb.append(tb)

        # ---- main loop: groups of G tiles emitted stage-major so every
        # engine always has G independent work items in flight ----
        G = 4
        for g0 in range(0, ntiles, G):
            grp = list(range(g0, min(g0 + G, ntiles)))
            xt_g, xsq_g, m8_g, idx_g, outq_g, resid_g = {}, {}, {}, {}, {}, {}
            for tix in grp:
                b, hb = divmod(tix, NHB)
                xt = xt_pool.tile([128, 2 * 128], F32, tag="xt")
                for j in range(2):
                    nc.sync.dma_start(xt[:, 128 * j:128 * (j + 1)],
                                      xr[b, hb][128 * j:128 * (j + 1), :])
                xt_g[tix] = xt
                resid_g[tix] = xt
                xsq = small_pool.tile([128, 2], F32, tag="xsq")
                for j in range(2):
                    tr = scratch_pool.tile([128, 128], F32, tag="sqtrash")
                    nc.scalar.activation(
                        tr[:], xt[:, 128 * j:128 * (j + 1)],
                        AF.Square, accum_out=xsq[:, j:j + 1])
                xsq_g[tix] = xsq
                m8_g[tix] = m8_pool.tile([128, 8 * NCB], F32, tag="m8", name=f"m8_{tix}")
                idx_g[tix] = idx_pool.tile([128, NCB], U32, tag="idxs", name=f"idxs_{tix}")
                outq_g[tix] = outq_pool.tile([128, D], F32, tag="outq", name=f"outq_{tix}")

            for q in range(NCB):
                rh_g, rl_g, ps_g = {}, {}, {}
                for tix in grp:
                    # fp16 split of residual: rh = fp16(r); rl = fp16(r - rh)
                    rh = rh_pool.tile([128, 2 * 128], F16, tag="rh")
                    nc.scalar.copy(rh[:], resid_g[tix][:])
                    rl = rl_pool.tile([128, 2 * 128], F16, tag="rl")
                    nc.gpsimd.tensor_tensor(rl[:], resid_g[tix][:], rh[:],
                                            op=AX.subtract)
                    rh_g[tix], rl_g[tix] = rh, rl
                for tix in grp:
                    # scores = bias + rh.ch + rh.cl + rl.ch  (PSUM accumulate)
                    ps = ps_scores.tile([128, K], F32, tag="scores")
                    nc.tensor.matmul(ps[:], ones3[:], bias_sb[q][:],
                                     start=True, stop=False)
                    for j in range(2):
                        sl = slice(128 * j, 128 * (j + 1))
                        nc.tensor.matmul(ps[:], rh_g[tix][:, sl], cbh_sb[q][j][:],
                                         start=False, stop=False)
                        nc.tensor.matmul(ps[:], rh_g[tix][:, sl], cbl_sb[q][j][:],
                                         start=False, stop=False)
                        nc.tensor.matmul(ps[:], rl_g[tix][:, sl], cbh_sb[q][j][:],
                                         start=False, stop=(j == 1))
                    ps_g[tix] = ps
                    # max + argmax (STT overwrites the dead scores bank in place)
                    m8, idxs = m8_g[tix], idx_g[tix]
                    nc.vector.max(m8[:, 8 * q:8 * q + 8], ps[:])
                    idxf = small_pool.tile([128, 1], F32, tag="idxf")
                    nc.vector.scalar_tensor_tensor(
                        ps[:], ps[:], m8[:, 8 * q:8 * q + 1], iota_f[:],
                        op0=AX.is_equal, op1=AX.mult, accum_out=idxf[:])
                    nc.vector.tensor_scalar_min(idxf[:], idxf[:], float(K - 1))
                    nc.vector.tensor_copy(idxs[:, q:q + 1], idxf[:])
                    # gather codebook rows; accumulates into output across stages
                    nc.gpsimd.indirect_dma_start(
                        out=outq_g[tix][:],
                        out_offset=None,
                        in_=cbs[q].ap()[:],
                        in_offset=IndirectOffsetOnAxis(ap=idxs[:, q:q + 1], axis=0),
                        compute_op=(AX.bypass if q == 0 else AX.add))
                if q < NCB - 1:
                    for tix in grp:
                        # residT = xT - transpose(out_acc)
                        tp = ps_tp.tile([128, 2 * 128], F32, tag="tp")
                        for j in range(2):
                            nc.tensor.transpose(
                                tp[:, 128 * j:128 * (j + 1)],
                                outq_g[tix][:, 128 * j:128 * (j + 1)], identity[:])
                        resid_new = resid_pool.tile([128, 2 * 128], F32, tag="resid")
                        nc.vector.tensor_tensor(
                            resid_new[:], xt_g[tix][:], tp[:], op=AX.subtract)
                        resid_g[tix] = resid_new

            for tix in grp:
                b, hb = divmod(tix, NHB)
                nc.sync.dma_start(oq[b, hb], outq_g[tix][:])
                nc.sync.dma_start(oi[b, hb], idx_g[tix][:])
                nc.sync.dma_start(out_m.ap()[tix], m8_g[tix][:])
                nc.sync.dma_start(out_xsq.ap()[tix], xsq_g[tix][:])

    nc.compile()
    return nc


def _split16(a):
    """exact 2-way fp16 split: a ~= h + l (h,l fp16)"""
    h = a.astype(np.float16)
    l = (a.astype(np.float64) - h.astype(np.float64)).astype(np.float16)
    return h, l


def _prep_host(encoded_x, codebooks):
    x = np.ascontiguousarray(np.asarray(encoded_x), dtype=np.float32)
    cb = np.ascontiguousarray(np.asarray(codebooks), dtype=np.float32)
    cbt = np.ascontiguousarray(np.transpose(cb, (0, 2, 1)))       # [4, 256, 512]
    ch, cl = _split16(cbt)
    b64 = -0.5 * (cb.astype(np.float64) ** 2).sum(-1)             # [4, 512]
    b_h = b64.astype(np.float16)
    r1 = b64 - b_h.astype(np.float64)
    b_m = r1.astype(np.float16)
    r2 = r1 - b_m.astype(np.float64)
    b_l = r2.astype(np.float16)
    bias3 = np.ascontiguousarray(np.stack([b_h, b_m, b_l], axis=1))  # [4, 3, 512]
    in_maps = []
    for i in range(NCORES):
        m = {"x": np.ascontiguousarray(x[i * BPC:(i + 1) * BPC]),
             "cbh": ch, "cbl": cl, "bias3": bias3}
        for q in range(NCB):
            m[f"cb{q}"] = cb[q]
        in_maps.append(m)
    return in_maps


def _assemble(results):
    qo = np.concatenate([results[i]["out_q"] for i in range(NCORES)], axis=0)
    idx = np.concatenate([results[i]["out_i"] for i in range(NCORES)],
                         axis=0).astype(np.int32)
    sum_xsq = sum(results[i]["out_xsq"].sum(dtype=np.float64) for i in range(NCORES))
    summax = [sum(results[i]["out_m"][:, :, 8 * q].sum(dtype=np.float64)
                  for i in range(NCORES)) for q in range(NCB)]
    n_el = float(B * H * W) * D
    losses, csum = [], 0.0
    for q in range(NCB):
        csum += summax[q]
        losses.append((sum_xsq - 2.0 * csum) / n_el)
    loss = np.array(losses, dtype=np.float32)
    return qo, idx, loss


_CACHED_NC = None


def kernel(encoded_x, codebooks):
    global _CACHED_NC
    if _CACHED_NC is None:
        _CACHED_NC = build_kernel()
    in_maps = _prep_host(encoded_x, codebooks)
    res = bass_utils.run_bass_kernel_spmd(_CACHED_NC, in_maps,
                                          core_ids=list(range(NCORES)))
    return _assemble(res.results)


# revision 21
# speedup vs baseline: 3.7072x; 1.4166x over previous
"""RVQ (4-stage residual vector quantization) Trainium2 Bass kernel.

Problem (hardcoded): encoded_x [16, 256, 1024, 4] f32, codebooks [4, 512, 256] f32.
reference: x_in = transpose(encoded_x, (0,3,2,1)).reshape(-1, 256)  (N=65536 tokens)
4 stages of: d = ||r||^2 - 2 r.c + ||c||^2 ; idx = argmin_k d ; quant = c[idx];
out += quant ; r -= quant.  Outputs: (quantized [N,256] f32, indices [N,4] i32, loss [4] f32).

Sharding: batch dim (16) split across 8 cores (2 each; 8192 tokens/core).

Numerics: argmin needs fp32-class scores (min top-2 gap ~5e-5 vs typical score ~40).
fp32 PE matmul is 4 cyc/row; instead scores are computed with an exact fp16
double-double split at 1 cyc/row:
    r = rh + rl (fp16 pairs), c = ch + cl (fp16 pairs, host-precomputed)
    score = rh.ch + rh.cl + rl.ch + bias         (error ~2^-22, same flips as fp32)
    bias  = -0.5||c||^2 injected as a K=3 fp16 matmul (ones x 3-way fp16 split)

Device algorithm per 128-token tile (t = tokens on partitions):
  scores PSUM [128t, 512k] = 7 fp16 matmuls (1 bias + 6 split chunks)
  m = max_k scores                (DVE max8, PSUM src)
  idx = sum_k (scores==m)*iota_k  (DVE scalar_tensor_tensor with accum_out)
  gather codebook rows via indirect DMA (accumulating into the output tile across stages)
  residual: residT = xT - transpose(out_acc) (PE transpose + DVE subtract), then
  rh (ACT cast) / rl (GPSIMD subtract, fp16 out)
  loss via d_min = ||r||^2 - 2m => needs only sum(x^2) and per-token maxes (host-combined)
"""

import numpy as np
from contextlib import ExitStack

import concourse.bass as bass
import concourse.bacc as bacc
import concourse.tile as tile
from concourse import mybir, bass_utils
from concourse.bass import IndirectOffsetOnAxis
from concourse.masks import make_identity

B, C, H, W = 16, 256, 1024, 4
NCB, K, D = 4, 512, 256
NCORES = 8
BPC = B // NCORES              # batches per core (2)
HBLK = 32                      # h rows per tile -> 32*4 = 128 tokens
NHB = H // HBLK                # 32 h-blocks
NTILES = BPC * NHB             # 64 tiles/core
TOKPC = BPC * H * W            # 8192 tokens/core

F32 = mybir.dt.float32
F16 = mybir.dt.float16
U32 = mybir.dt.uint32
I32 = mybir.dt.int32
AX = mybir.AluOpType
AF = mybir.ActivationFunctionType


def build_kernel(ntiles=NTILES):
    nc = bacc.Bacc("TRN2", target_bir_lowering=False, debug=False, num_devices=NCORES)
    x = nc.dram_tensor("x", [BPC, C, H, W], F32, kind="ExternalInput")
    cbh = nc.dram_tensor("cbh", [NCB, D, K], F16, kind="ExternalInput")
    cbl = nc.dram_tensor("cbl", [NCB, D, K], F16, kind="ExternalInput")
    cbs = [nc.dram_tensor(f"cb{q}", [K, D], F32, kind="ExternalInput") for q in range(NCB)]
    bias3 = nc.dram_tensor("bias3", [NCB, 3, K], F16, kind="ExternalInput")
    out_q = nc.dram_tensor("out_q", [TOKPC, D], F32, kind="ExternalOutput")
    out_i = nc.dram_tensor("out_i", [TOKPC, NCB], U32, kind="ExternalOutput")
    out_m = nc.dram_tensor("out_m", [NTILES, 128, 8 * NCB], F32, kind="ExternalOutput")
    out_xsq = nc.dram_tensor("out_xsq", [NTILES, 128, 2], F32, kind="ExternalOutput")

    # token tile (b, hb): tokens t_local = b*4096 + w*1024 + hb*32 + hh ; partition p = hh*4 + w
    xr = x.ap().rearrange("b c (hb hh) w -> b hb c (hh w)", hh=HBLK)
    oq = out_q.ap().rearrange("(b w hb hh) c -> b hb hh w c", w=W, hb=NHB, hh=HBLK)
    oi = out_i.ap().rearrange("(b w hb hh) c -> b hb hh w c", w=W, hb=NHB, hh=HBLK)

    with tile.TileContext(nc) as tc, ExitStack() as ctx:
        const = ctx.enter_context(tc.tile_pool(name="const", bufs=1))
        xt_pool = ctx.enter_context(tc.tile_pool(name="xt", bufs=17))
        resid_pool = ctx.enter_context(tc.tile_pool(name="resid", bufs=18))
        rh_pool = ctx.enter_context(tc.tile_pool(name="rh", bufs=17))
        rl_pool = ctx.enter_context(tc.tile_pool(name="rl", bufs=17))
        outq_pool = ctx.enter_context(tc.tile_pool(name="outq", bufs=17))
        m8_pool = ctx.enter_context(tc.tile_pool(name="m8", bufs=17))
        idx_pool = ctx.enter_context(tc.tile_pool(name="idx", bufs=17))
        small_pool = ctx.enter_context(tc.tile_pool(name="small", bufs=26))
        scratch_pool = ctx.enter_context(tc.tile_pool(name="scratch", bufs=4))
        ps_scores = ctx.enter_context(tc.tile_pool(name="ps_s", bufs=4, space="PSUM"))
        ps_tp = ctx.enter_context(tc.tile_pool(name="ps_tp", bufs=4, space="PSUM"))

        # ---- static tiles ----
        identity = const.tile([128, 128], F32, tag="ident")
        make_identity(nc, identity[:])
        ones3 = const.tile([3, 128], F16, tag="ones3")
        nc.gpsimd.memset(ones3[:], 1.0)
        iota_i = const.tile([128, K], I32, tag="iota_i")
        nc.gpsimd.iota(iota_i[:], pattern=[[1, K]], base=0, channel_multiplier=0)
        iota_f = const.tile([128, K], F32, tag="iota_f")
        nc.vector.tensor_copy(iota_f[:], iota_i[:])

        cbh_sb, cbl_sb, bias_sb = [], [], []
        for q in range(NCB):
            hs, ls = [], []
            for j in range(2):
                th = const.tile([128, K], F16, tag=f"cbh{q}_{j}")
                nc.sync.dma_start(th[:], cbh.ap()[q, 128 * j:128 * (j + 1), :])
                hs.append(th)
                tl = const.tile([128, K], F16, tag=f"cbl{q}_{j}")
                nc.sync.dma_start(tl[:], cbl.ap()[q, 128 * j:128 * (j + 1), :])
                ls.append(tl)
            cbh_sb.append(hs)
            cbl_sb.append(ls)
            tb = const.tile([3, K], F16, tag=f"bias{q}")
            nc.sync.dma_start(tb[:], bias3.ap()[q])
            bias_sb.append(tb)

        # ---- main loop: groups of G tiles emitted stage-major so every
        # engine always has G independent work items in flight ----
        G = 8
        for g0 in range(0, ntiles, G):
            grp = list(range(g0, min(g0 + G, ntiles)))
            xt_g, xsq_g, m8_g, idx_g, outq_g, resid_g = {}, {}, {}, {}, {}, {}
            for tix in grp:
                b, hb = divmod(tix, NHB)
                xt = xt_pool.tile([128, 2 * 128], F32, tag="xt")
                for j in range(2):
                    nc.sync.dma_start(xt[:, 128 * j:128 * (j + 1)],
                                      xr[b, hb][128 * j:128 * (j + 1), :])
                xt_g[tix] = xt
                resid_g[tix] = xt
                xsq = small_pool.tile([128, 2], F32, tag="xsq")
                for j in range(2):
                    tr = scratch_pool.tile([128, 128], F32, tag="sqtrash")
                    nc.scalar.activation(
                        tr[:], xt[:, 128 * j:128 * (j + 1)],
                        AF.Square, accum_out=xsq[:, j:j + 1])
                xsq_g[tix] = xsq
                m8_g[tix] = m8_pool.tile([128, 8 * NCB], F32, tag="m8", name=f"m8_{tix}")
                idx_g[tix] = idx_pool.tile([128, NCB], U32, tag="idxs", name=f"idxs_{tix}")
                outq_g[tix] = outq_pool.tile([128, D], F32, tag="outq", name=f"outq_{tix}")

            for q in range(NCB):
                rh_g, rl_g, ps_g = {}, {}, {}
                for tix in grp:
                    # fp16 split of residual: rh = fp16(r); rl = fp16(r - rh)
                    rh = rh_pool.tile([128, 2 * 128], F16, tag="rh")
                    nc.scalar.copy(rh[:], resid_g[tix][:])
                    rl = rl_pool.tile([128, 2 * 128], F16, tag="rl")
                    nc.vector.tensor_tensor(rl[:], resid_g[tix][:], rh[:],
                                            op=AX.subtract)
                    rh_g[tix], rl_g[tix] = rh, rl
                for tix in grp:
                    # scores = bias + rh.ch + rh.cl + rl.ch  (PSUM accumulate)
                    ps = ps_scores.tile([128, K], F32, tag="scores")
                    nc.tensor.matmul(ps[:], ones3[:], bias_sb[q][:],
                                     start=True, stop=False)
                    for j in range(2):
                        sl = slice(128 * j, 128 * (j + 1))
                        nc.tensor.matmul(ps[:], rh_g[tix][:, sl], cbh_sb[q][j][:],
                                         start=False, stop=False)
                        nc.tensor.matmul(ps[:], rh_g[tix][:, sl], cbl_sb[q][j][:],
                                         start=False, stop=False)
                        nc.tensor.matmul(ps[:], rl_g[tix][:, sl], cbh_sb[q][j][:],
                                         start=False, stop=(j == 1))
                    ps_g[tix] = ps
                    # max + argmax (STT overwrites the dead scores bank in place)
                    m8, idxs = m8_g[tix], idx_g[tix]
                    nc.vector.max(m8[:, 8 * q:8 * q + 8], ps[:])
                    idxf = small_pool.tile([128, 1], F32, tag="idxf")
                    nc.vector.scalar_tensor_tensor(
                        ps[:], ps[:], m8[:, 8 * q:8 * q + 1], iota_f[:],
                        op0=AX.is_equal, op1=AX.mult, accum_out=idxf[:])
                    nc.vector.tensor_scalar_min(idxf[:], idxf[:], float(K - 1))
                    nc.vector.tensor_copy(idxs[:, q:q + 1], idxf[:])
                    # gather codebook rows; accumulates into output across stages
                    nc.gpsimd.indirect_dma_start(
                        out=outq_g[tix][:],
                        out_offset=None,
                        in_=cbs[q].ap()[:],
                        in_offset=IndirectOffsetOnAxis(ap=idxs[:, q:q + 1], axis=0),
                        compute_op=(AX.bypass if q == 0 else AX.add))
                if q < NCB - 1:
                    for tix in grp:
                        # residT = xT - transpose(out_acc)
                        tp = ps_tp.tile([128, 2 * 128], F32, tag="tp")
                        for j in range(2):
                            nc.tensor.transpose(
                                tp[:, 128 * j:128 * (j + 1)],
                                outq_g[tix][:, 128 * j:128 * (j + 1)], identity[:])
                        resid_new = resid_pool.tile([128, 2 * 128], F32, tag="resid")
                        nc.vector.tensor_tensor(
                            resid_new[:], xt_g[tix][:], tp[:], op=AX.subtract)
                        resid_g[tix] = resid_new

            for tix in grp:
                b, hb = divmod(tix, NHB)
                nc.sync.dma_start(oq[b, hb], outq_g[tix][:])
                nc.sync.dma_start(oi[b, hb], idx_g[tix][:])
                nc.sync.dma_start(out_m.ap()[tix], m8_g[tix][:])
                nc.sync.dma_start(out_xsq.ap()[tix], xsq_g[tix][:])

    nc.compile()
    return nc


def _split16(a):
    """exact 2-way fp16 split: a ~= h + l (h,l fp16)"""
    h = a.astype(np.float16)
    l = (a.astype(np.float64) - h.astype(np.float64)).astype(np.float16)
    return h, l


def _prep_host(encoded_x, codebooks):
    x = np.ascontiguousarray(np.asarray(encoded_x), dtype=np.float32)
    cb = np.ascontiguousarray(np.asarray(codebooks), dtype=np.float32)
    cbt = np.ascontiguousarray(np.transpose(cb, (0, 2, 1)))       # [4, 256, 512]
    ch, cl = _split16(cbt)
    b64 = -0.5 * (cb.astype(np.float64) ** 2).sum(-1)             # [4, 512]
    b_h = b64.astype(np.float16)
    r1 = b64 - b_h.astype(np.float64)
    b_m = r1.astype(np.float16)
    r2 = r1 - b_m.astype(np.float64)
    b_l = r2.astype(np.float16)
    bias3 = np.ascontiguousarray(np.stack([b_h, b_m, b_l], axis=1))  # [4, 3, 512]
    in_maps = []
    for i in range(NCORES):
        m = {"x": np.ascontiguousarray(x[i * BPC:(i + 1) * BPC]),
             "cbh": ch, "cbl": cl, "bias3": bias3}
        for q in range(NCB):
            m[f"cb{q}"] = cb[q]
        in_maps.append(m)
    return in_maps


def _assemble(results):
    qo = np.concatenate([results[i]["out_q"] for i in range(NCORES)], axis=0)
    idx = np.concatenate([results[i]["out_i"] for i in range(NCORES)],
                         axis=0).astype(np.int32)
    sum_xsq = sum(results[i]["out_xsq"].sum(dtype=np.float64) for i in range(NCORES))
    summax = [sum(results[i]["out_m"][:, :, 8 * q].sum(dtype=np.float64)
                  for i in range(NCORES)) for q in range(NCB)]
    n_el = float(B * H * W) * D
    losses, csum = [], 0.0
    for q in range(NCB):
        csum += summax[q]
        losses.append((sum_xsq - 2.0 * csum) / n_el)
    loss = np.array(losses, dtype=np.float32)
    return qo, idx, loss


_CACHED_NC = None


def kernel(encoded_x, codebooks):
    global _CACHED_NC
    if _CACHED_NC is None:
        _CACHED_NC = build_kernel()
    in_maps = _prep_host(encoded_x, codebooks)
    res = bass_utils.run_bass_kernel_spmd(_CACHED_NC, in_maps,
                                          core_ids=list(range(NCORES)))
    return _assemble(res.results)


# revision 22
# speedup vs baseline: 3.8041x; 1.0261x over previous
"""RVQ (4-stage residual vector quantization) Trainium2 Bass kernel.

Problem (hardcoded): encoded_x [16, 256, 1024, 4] f32, codebooks [4, 512, 256] f32.
reference: x_in = transpose(encoded_x, (0,3,2,1)).reshape(-1, 256)  (N=65536 tokens)
4 stages of: d = ||r||^2 - 2 r.c + ||c||^2 ; idx = argmin_k d ; quant = c[idx];
out += quant ; r -= quant.  Outputs: (quantized [N,256] f32, indices [N,4] i32, loss [4] f32).

Sharding: batch dim (16) split across 8 cores (2 each; 8192 tokens/core).

Numerics: argmin needs fp32-class scores (min top-2 gap ~5e-5 vs typical score ~40).
fp32 PE matmul is 4 cyc/row; instead scores are computed with an exact fp16
double-double split at 1 cyc/row:
    r = rh + rl (fp16 pairs), c = ch + cl (fp16 pairs, host-precomputed)
    score = rh.ch + rh.cl + rl.ch + bias         (error ~2^-22, same flips as fp32)
    bias  = -0.5||c||^2 injected as a K=3 fp16 matmul (ones x 3-way fp16 split)

Device algorithm per 128-token tile (t = tokens on partitions):
  scores PSUM [128t, 512k] = 7 fp16 matmuls (1 bias + 6 split chunks)
  m = max_k scores                (DVE max8, PSUM src)
  idx = sum_k (scores==m)*iota_k  (DVE scalar_tensor_tensor with accum_out)
  gather codebook rows via indirect DMA (accumulating into the output tile across stages)
  residual: residT = xT - transpose(out_acc) (PE transpose + DVE subtract), then
  rh (ACT cast) / rl (GPSIMD subtract, fp16 out)
  loss via d_min = ||r||^2 - 2m => needs only sum(x^2) and per-token maxes (host-combined)
"""

import numpy as np
from contextlib import ExitStack

import concourse.bass as bass
import concourse.bacc as bacc
import concourse.tile as tile
from concourse import mybir, bass_utils
from concourse.bass import IndirectOffsetOnAxis
from concourse.masks import make_identity

B, C, H, W = 16, 256, 1024, 4
NCB, K, D = 4, 512, 256
NCORES = 8
BPC = B // NCORES              # batches per core (2)
HBLK = 32                      # h rows per tile -> 32*4 = 128 tokens
NHB = H // HBLK                # 32 h-blocks
NTILES = BPC * NHB             # 64 tiles/core
TOKPC = BPC * H * W            # 8192 tokens/core

F32 = mybir.dt.float32
F16 = mybir.dt.float16
U32 = mybir.dt.uint32
I32 = mybir.dt.int32
AX = mybir.AluOpType
AF = mybir.ActivationFunctionType


def build_kernel(ntiles=NTILES):
    nc = bacc.Bacc("TRN2", target_bir_lowering=False, debug=False, num_devices=NCORES)
    x = nc.dram_tensor("x", [BPC, C, H, W], F32, kind="ExternalInput")
    cbh = nc.dram_tensor("cbh", [NCB, D, K], F16, kind="ExternalInput")
    cbl = nc.dram_tensor("cbl", [NCB, D, K], F16, kind="ExternalInput")
    cbs = [nc.dram_tensor(f"cb{q}", [K, D], F32, kind="ExternalInput") for q in range(NCB)]
    bias3 = nc.dram_tensor("bias3", [NCB, 3, K], F16, kind="ExternalInput")
    out_q = nc.dram_tensor("out_q", [TOKPC, D], F32, kind="ExternalOutput")
    out_i = nc.dram_tensor("out_i", [TOKPC, NCB], U32, kind="ExternalOutput")
    out_m = nc.dram_tensor("out_m", [NTILES, 128, 8 * NCB], F32, kind="ExternalOutput")
    out_xsq = nc.dram_tensor("out_xsq", [NTILES, 128, 2], F32, kind="ExternalOutput")

    # token tile (b, hb): tokens t_local = b*4096 + w*1024 + hb*32 + hh ; partition p = hh*4 + w
    xr = x.ap().rearrange("b c (hb hh) w -> b hb c (hh w)", hh=HBLK)
    oq = out_q.ap().rearrange("(b w hb hh) c -> b hb hh w c", w=W, hb=NHB, hh=HBLK)
    oi = out_i.ap().rearrange("(b w hb hh) c -> b hb hh w c", w=W, hb=NHB, hh=HBLK)

    with tile.TileContext(nc) as tc, ExitStack() as ctx:
        const = ctx.enter_context(tc.tile_pool(name="const", bufs=1))
        xt_pool = ctx.enter_context(tc.tile_pool(name="xt", bufs=25))
        resid_pool = ctx.enter_context(tc.tile_pool(name="resid", bufs=26))
        rh_pool = ctx.enter_context(tc.tile_pool(name="rh", bufs=25))
        rl_pool = ctx.enter_context(tc.tile_pool(name="rl", bufs=25))
        outq_pool = ctx.enter_context(tc.tile_pool(name="outq", bufs=25))
        m8_pool = ctx.enter_context(tc.tile_pool(name="m8", bufs=25))
        idx_pool = ctx.enter_context(tc.tile_pool(name="idx", bufs=25))
        small_pool = ctx.enter_context(tc.tile_pool(name="small", bufs=38))
        scratch_pool = ctx.enter_context(tc.tile_pool(name="scratch", bufs=4))
        ps_scores = ctx.enter_context(tc.tile_pool(name="ps_s", bufs=5, space="PSUM"))
        ps_tp = ctx.enter_context(tc.tile_pool(name="ps_tp", bufs=3, space="PSUM"))

        # ---- static tiles ----
        identity = const.tile([128, 128], F32, tag="ident")
        make_identity(nc, identity[:])
        ones3 = const.tile([3, 128], F16, tag="ones3")
        nc.gpsimd.memset(ones3[:], 1.0)
        iota_i = const.tile([128, K], I32, tag="iota_i")
        nc.gpsimd.iota(iota_i[:], pattern=[[1, K]], base=0, channel_multiplier=0)
        iota_f = const.tile([128, K], F32, tag="iota_f")
        nc.vector.tensor_copy(iota_f[:], iota_i[:])

        cbh_sb, cbl_sb, bias_sb = [], [], []
        for q in range(NCB):
            hs, ls = [], []
            for j in range(2):
                th = const.tile([128, K], F16, tag=f"cbh{q}_{j}")
                nc.sync.dma_start(th[:], cbh.ap()[q, 128 * j:128 * (j + 1), :])
                hs.append(th)
                tl = const.tile([128, K], F16, tag=f"cbl{q}_{j}")
                nc.sync.dma_start(tl[:], cbl.ap()[q, 128 * j:128 * (j + 1), :])
                ls.append(tl)
            cbh_sb.append(hs)
            cbl_sb.append(ls)
            tb = const.tile([3, K], F16, tag=f"bias{q}")
            nc.sync.dma_start(tb[:], bias3.ap()[q])
            bias_sb.append(tb)

        # ---- main loop: groups of G tiles emitted stage-major so every
        # engine always has G independent work items in flight ----
        G = 12
        for g0 in range(0, ntiles, G):
            grp = list(range(g0, min(g0 + G, ntiles)))
            xt_g, xsq_g, m8_g, idx_g, outq_g, resid_g = {}, {}, {}, {}, {}, {}
            for tix in grp:
                b, hb = divmod(tix, NHB)
                xt = xt_pool.tile([128, 2 * 128], F32, tag="xt")
                for j in range(2):
                    nc.sync.dma_start(xt[:, 128 * j:128 * (j + 1)],
                                      xr[b, hb][128 * j:128 * (j + 1), :])
                xt_g[tix] = xt
                resid_g[tix] = xt
                xsq = small_pool.tile([128, 2], F32, tag="xsq")
                for j in range(2):
                    tr = scratch_pool.tile([128, 128], F32, tag="sqtrash")
                    nc.scalar.activation(
                        tr[:], xt[:, 128 * j:128 * (j + 1)],
                        AF.Square, accum_out=xsq[:, j:j + 1])
                xsq_g[tix] = xsq
                m8_g[tix] = m8_pool.tile([128, 8 * NCB], F32, tag="m8", name=f"m8_{tix}")
                idx_g[tix] = idx_pool.tile([128, NCB], U32, tag="idxs", name=f"idxs_{tix}")
                outq_g[tix] = outq_pool.tile([128, D], F32, tag="outq", name=f"outq_{tix}")

            for q in range(NCB):
                rh_g, rl_g, ps_g = {}, {}, {}
                for tix in grp:
                    # fp16 split of residual: rh = fp16(r); rl = fp16(r - rh)
                    rh = rh_pool.tile([128, 2 * 128], F16, tag="rh")
                    nc.scalar.copy(rh[:], resid_g[tix][:])
                    rl = rl_pool.tile([128, 2 * 128], F16, tag="rl")
                    nc.vector.tensor_tensor(rl[:], resid_g[tix][:], rh[:],
                                            op=AX.subtract)
                    rh_g[tix], rl_g[tix] = rh, rl
                for tix in grp:
                    # scores = bias + rh.ch + rh.cl + rl.ch  (PSUM accumulate)
                    ps = ps_scores.tile([128, K], F32, tag="scores")
                    nc.tensor.matmul(ps[:], ones3[:], bias_sb[q][:],
                                     start=True, stop=False)
                    for j in range(2):
                        sl = slice(128 * j, 128 * (j + 1))
                        nc.tensor.matmul(ps[:], rh_g[tix][:, sl], cbh_sb[q][j][:],
                                         start=False, stop=False)
                        nc.tensor.matmul(ps[:], rh_g[tix][:, sl], cbl_sb[q][j][:],
                                         start=False, stop=False)
                        nc.tensor.matmul(ps[:], rl_g[tix][:, sl], cbh_sb[q][j][:],
                                         start=False, stop=(j == 1))
                    ps_g[tix] = ps
                    # max + argmax (STT overwrites the dead scores bank in place)
                    m8, idxs = m8_g[tix], idx_g[tix]
                    nc.vector.max(m8[:, 8 * q:8 * q + 8], ps[:])
                    idxf = small_pool.tile([128, 1], F32, tag="idxf")
                    nc.vector.scalar_tensor_tensor(
                        ps[:], ps[:], m8[:, 8 * q:8 * q + 1], iota_f[:],
                        op0=AX.is_equal, op1=AX.mult, accum_out=idxf[:])
                    nc.vector.tensor_scalar_min(idxf[:], idxf[:], float(K - 1))
                    nc.vector.tensor_copy(idxs[:, q:q + 1], idxf[:])
                    # gather codebook rows; accumulates into output across stages
                    nc.gpsimd.indirect_dma_start(
                        out=outq_g[tix][:],
                        out_offset=None,
                        in_=cbs[q].ap()[:],
                        in_offset=IndirectOffsetOnAxis(ap=idxs[:, q:q + 1], axis=0),
                        compute_op=(AX.bypass if q == 0 else AX.add))
                if q < NCB - 1:
                    for tix in grp:
                        # residT = xT - transpose(out_acc)
                        tp = ps_tp.tile([128, 2 * 128], F32, tag="tp")
                        for j in range(2):
                            nc.tensor.transpose(
                                tp[:, 128 * j:128 * (j + 1)],
                                outq_g[tix][:, 128 * j:128 * (j + 1)], identity[:])
                        resid_new = resid_pool.tile([128, 2 * 128], F32, tag="resid")
                        nc.vector.tensor_tensor(
                            resid_new[:], xt_g[tix][:], tp[:], op=AX.subtract)
                        resid_g[tix] = resid_new

            for tix in grp:
                b, hb = divmod(tix, NHB)
                nc.sync.dma_start(oq[b, hb], outq_g[tix][:])
                nc.sync.dma_start(oi[b, hb], idx_g[tix][:])
                nc.sync.dma_start(out_m.ap()[tix], m8_g[tix][:])
                nc.sync.dma_start(out_xsq.ap()[tix], xsq_g[tix][:])

    nc.compile()
    return nc


def _split16(a):
    """exact 2-way fp16 split: a ~= h + l (h,l fp16)"""
    h = a.astype(np.float16)
    l = (a.astype(np.float64) - h.astype(np.float64)).astype(np.float16)
    return h, l


def _prep_host(encoded_x, codebooks):
    x = np.ascontiguousarray(np.asarray(encoded_x), dtype=np.float32)
    cb = np.ascontiguousarray(np.asarray(codebooks), dtype=np.float32)
    cbt = np.ascontiguousarray(np.transpose(cb, (0, 2, 1)))       # [4, 256, 512]
    ch, cl = _split16(cbt)
    b64 = -0.5 * (cb.astype(np.float64) ** 2).sum(-1)             # [4, 512]
    b_h = b64.astype(np.float16)
    r1 = b64 - b_h.astype(np.float64)
    b_m = r1.astype(np.float16)
    r2 = r1 - b_m.astype(np.float64)
    b_l = r2.astype(np.float16)
    bias3 = np.ascontiguousarray(np.stack([b_h, b_m, b_l], axis=1))  # [4, 3, 512]
    in_maps = []
    for i in range(NCORES):
        m = {"x": np.ascontiguousarray(x[i * BPC:(i + 1) * BPC]),
             "cbh": ch, "cbl": cl, "bias3": bias3}
        for q in range(NCB):
            m[f"cb{q}"] = cb[q]
        in_maps.append(m)
    return in_maps


def _assemble(results):
    qo = np.concatenate([results[i]["out_q"] for i in range(NCORES)], axis=0)
    idx = np.concatenate([results[i]["out_i"] for i in range(NCORES)],
                         axis=0).astype(np.int32)
    sum_xsq = sum(results[i]["out_xsq"].sum(dtype=np.float64) for i in range(NCORES))
    summax = [sum(results[i]["out_m"][:, :, 8 * q].sum(dtype=np.float64)
                  for i in range(NCORES)) for q in range(NCB)]
    n_el = float(B * H * W) * D
    losses, csum = [], 0.0
    for q in range(NCB):
        csum += summax[q]
        losses.append((sum_xsq - 2.0 * csum) / n_el)
    loss = np.array(losses, dtype=np.float32)
    return qo, idx, loss


_CACHED_NC = None


def kernel(encoded_x, codebooks):
    global _CACHED_NC
    if _CACHED_NC is None:
        _CACHED_NC = build_kernel()
    in_maps = _prep_host(encoded_x, codebooks)
    res = bass_utils.run_bass_kernel_spmd(_CACHED_NC, in_maps,
                                          core_ids=list(range(NCORES)))
    return _assemble(res.results)


# revision 23
# speedup vs baseline: 3.8636x; 1.0157x over previous
"""RVQ (4-stage residual vector quantization) Trainium2 Bass kernel.

Problem (hardcoded): encoded_x [16, 256, 1024, 4] f32, codebooks [4, 512, 256] f32.
reference: x_in = transpose(encoded_x, (0,3,2,1)).reshape(-1, 256)  (N=65536 tokens)
4 stages of: d = ||r||^2 - 2 r.c + ||c||^2 ; idx = argmin_k d ; quant = c[idx];
out += quant ; r -= quant.  Outputs: (quantized [N,256] f32, indices [N,4] i32, loss [4] f32).

Sharding: batch dim (16) split across 8 cores (2 each; 8192 tokens/core).

Numerics: argmin needs fp32-class scores (min top-2 gap ~5e-5 vs typical score ~40).
fp32 PE matmul is 4 cyc/row; instead scores are computed with an exact fp16
double-double split at 1 cyc/row:
    r = rh + rl (fp16 pairs), c = ch + cl (fp16 pairs, host-precomputed)
    score = rh.ch + rh.cl + rl.ch + bias         (error ~2^-22, same flips as fp32)
    bias  = -0.5||c||^2 injected as a K=3 fp16 matmul (ones x 3-way fp16 split)

Device algorithm per 128-token tile (t = tokens on partitions):
  scores PSUM [128t, 512k] = 7 fp16 matmuls (1 bias + 6 split chunks)
  m = max_k scores                (DVE max8, PSUM src)
  idx = sum_k (scores==m)*iota_k  (DVE scalar_tensor_tensor with accum_out)
  gather codebook rows via indirect DMA (accumulating into the output tile across stages)
  residual: residT = xT - transpose(out_acc) (PE transpose + DVE subtract), then
  rh (ACT cast) / rl (GPSIMD subtract, fp16 out)
  loss via d_min = ||r||^2 - 2m => needs only sum(x^2) and per-token maxes (host-combined)
"""

import numpy as np
from contextlib import ExitStack

import concourse.bass as bass
import concourse.bacc as bacc
import concourse.tile as tile
from concourse import mybir, bass_utils
from concourse.bass import IndirectOffsetOnAxis
from concourse.masks import make_identity

B, C, H, W = 16, 256, 1024, 4
NCB, K, D = 4, 512, 256
NCORES = 8
BPC = B // NCORES              # batches per core (2)
HBLK = 32                      # h rows per tile -> 32*4 = 128 tokens
NHB = H // HBLK                # 32 h-blocks
NTILES = BPC * NHB             # 64 tiles/core
TOKPC = BPC * H * W            # 8192 tokens/core

F32 = mybir.dt.float32
F16 = mybir.dt.float16
U32 = mybir.dt.uint32
I32 = mybir.dt.int32
AX = mybir.AluOpType
AF = mybir.ActivationFunctionType


def build_kernel(ntiles=NTILES):
    nc = bacc.Bacc("TRN2", target_bir_lowering=False, debug=False, num_devices=NCORES)
    x = nc.dram_tensor("x", [BPC, C, H, W], F32, kind="ExternalInput")
    cbh = nc.dram_tensor("cbh", [NCB, D, K], F16, kind="ExternalInput")
    cbl = nc.dram_tensor("cbl", [NCB, D, K], F16, kind="ExternalInput")
    cbs = [nc.dram_tensor(f"cb{q}", [K, D], F32, kind="ExternalInput") for q in range(NCB)]
    bias3 = nc.dram_tensor("bias3", [NCB, 3, K], F16, kind="ExternalInput")
    out_q = nc.dram_tensor("out_q", [TOKPC, D], F32, kind="ExternalOutput")
    out_i = nc.dram_tensor("out_i", [TOKPC, NCB], U32, kind="ExternalOutput")
    out_m = nc.dram_tensor("out_m", [NTILES, 128, 8 * NCB], F32, kind="ExternalOutput")
    out_xsq = nc.dram_tensor("out_xsq", [NTILES, 128, 2], F32, kind="ExternalOutput")

    # token tile (b, hb): tokens t_local = b*4096 + w*1024 + hb*32 + hh ; partition p = hh*4 + w
    xr = x.ap().rearrange("b c (hb hh) w -> b hb c (hh w)", hh=HBLK)
    oq = out_q.ap().rearrange("(b w hb hh) c -> b hb hh w c", w=W, hb=NHB, hh=HBLK)
    oi = out_i.ap().rearrange("(b w hb hh) c -> b hb hh w c", w=W, hb=NHB, hh=HBLK)

    with tile.TileContext(nc) as tc, ExitStack() as ctx:
        const = ctx.enter_context(tc.tile_pool(name="const", bufs=1))
        xt_pool = ctx.enter_context(tc.tile_pool(name="xt", bufs=25))
        resid_pool = ctx.enter_context(tc.tile_pool(name="resid", bufs=26))
        rh_pool = ctx.enter_context(tc.tile_pool(name="rh", bufs=25))
        rl_pool = ctx.enter_context(tc.tile_pool(name="rl", bufs=25))
        outq_pool = ctx.enter_context(tc.tile_pool(name="outq", bufs=25))
        m8_pool = ctx.enter_context(tc.tile_pool(name="m8", bufs=25))
        idx_pool = ctx.enter_context(tc.tile_pool(name="idx", bufs=25))
        small_pool = ctx.enter_context(tc.tile_pool(name="small", bufs=38))
        scratch_pool = ctx.enter_context(tc.tile_pool(name="scratch", bufs=4))
        ps_scores = ctx.enter_context(tc.tile_pool(name="ps_s", bufs=5, space="PSUM"))
        ps_tp = ctx.enter_context(tc.tile_pool(name="ps_tp", bufs=3, space="PSUM"))

        # ---- static tiles ----
        identity = const.tile([128, 128], F32, tag="ident")
        make_identity(nc, identity[:])
        ones3 = const.tile([3, 128], F16, tag="ones3")
        nc.gpsimd.memset(ones3[:], 1.0)
        iota_i = const.tile([128, K], I32, tag="iota_i")
        nc.gpsimd.iota(iota_i[:], pattern=[[1, K]], base=0, channel_multiplier=0)
        iota_f = const.tile([128, K], F32, tag="iota_f")
        nc.vector.tensor_copy(iota_f[:], iota_i[:])

        cbh_sb, cbl_sb, bias_sb = [], [], []
        for q in range(NCB):
            hs, ls = [], []
            for j in range(2):
                th = const.tile([128, K], F16, tag=f"cbh{q}_{j}")
                nc.sync.dma_start(th[:], cbh.ap()[q, 128 * j:128 * (j + 1), :])
                hs.append(th)
                tl = const.tile([128, K], F16, tag=f"cbl{q}_{j}")
                nc.sync.dma_start(tl[:], cbl.ap()[q, 128 * j:128 * (j + 1), :])
                ls.append(tl)
            cbh_sb.append(hs)
            cbl_sb.append(ls)
            tb = const.tile([3, K], F16, tag=f"bias{q}")
            nc.sync.dma_start(tb[:], bias3.ap()[q])
            bias_sb.append(tb)

        # ---- main loop: groups of G tiles emitted stage-major so every
        # engine always has G independent work items in flight ----
        G = 12
        for g0 in range(0, ntiles, G):
            grp = list(range(g0, min(g0 + G, ntiles)))
            xt_g, xsq_g, m8_g, idx_g, outq_g, resid_g = {}, {}, {}, {}, {}, {}
            for tix in grp:
                b, hb = divmod(tix, NHB)
                xt = xt_pool.tile([128, 2 * 128], F32, tag="xt")
                for j in range(2):
                    nc.scalar.dma_start(xt[:, 128 * j:128 * (j + 1)],
                                        xr[b, hb][128 * j:128 * (j + 1), :])
                xt_g[tix] = xt
                resid_g[tix] = xt
                xsq = small_pool.tile([128, 2], F32, tag="xsq")
                for j in range(2):
                    tr = scratch_pool.tile([128, 128], F32, tag="sqtrash")
                    nc.scalar.activation(
                        tr[:], xt[:, 128 * j:128 * (j + 1)],
                        AF.Square, accum_out=xsq[:, j:j + 1])
                xsq_g[tix] = xsq
                m8_g[tix] = m8_pool.tile([128, 8 * NCB], F32, tag="m8", name=f"m8_{tix}")
                idx_g[tix] = idx_pool.tile([128, NCB], U32, tag="idxs", name=f"idxs_{tix}")
                outq_g[tix] = outq_pool.tile([128, D], F32, tag="outq", name=f"outq_{tix}")

            for q in range(NCB):
                rh_g, rl_g, ps_g = {}, {}, {}
                for tix in grp:
                    # fp16 split of residual: rh = fp16(r); rl = fp16(r - rh)
                    rh = rh_pool.tile([128, 2 * 128], F16, tag="rh")
                    nc.scalar.copy(rh[:], resid_g[tix][:])
                    rl = rl_pool.tile([128, 2 * 128], F16, tag="rl")
                    nc.vector.tensor_tensor(rl[:], resid_g[tix][:], rh[:],
                                            op=AX.subtract)
                    rh_g[tix], rl_g[tix] = rh, rl
                for tix in grp:
                    # scores = bias + rh.ch + rh.cl + rl.ch  (PSUM accumulate)
                    ps = ps_scores.tile([128, K], F32, tag="scores")
                    nc.tensor.matmul(ps[:], ones3[:], bias_sb[q][:],
                                     start=True, stop=False)
                    for j in range(2):
                        sl = slice(128 * j, 128 * (j + 1))
                        nc.tensor.matmul(ps[:], rh_g[tix][:, sl], cbh_sb[q][j][:],
                                         start=False, stop=False)
                        nc.tensor.matmul(ps[:], rh_g[tix][:, sl], cbl_sb[q][j][:],
                                         start=False, stop=False)
                        nc.tensor.matmul(ps[:], rl_g[tix][:, sl], cbh_sb[q][j][:],
                                         start=False, stop=(j == 1))
                    ps_g[tix] = ps
                    # max + argmax (STT overwrites the dead scores bank in place)
                    m8, idxs = m8_g[tix], idx_g[tix]
                    nc.vector.max(m8[:, 8 * q:8 * q + 8], ps[:])
                    idxf = small_pool.tile([128, 1], F32, tag="idxf")
                    nc.vector.scalar_tensor_tensor(
                        ps[:], ps[:], m8[:, 8 * q:8 * q + 1], iota_f[:],
                        op0=AX.is_equal, op1=AX.mult, accum_out=idxf[:])
                    nc.vector.tensor_scalar_min(idxf[:], idxf[:], float(K - 1))
                    nc.vector.tensor_copy(idxs[:, q:q + 1], idxf[:])
                    # gather codebook rows; accumulates into output across stages
                    nc.gpsimd.indirect_dma_start(
                        out=outq_g[tix][:],
                        out_offset=None,
                        in_=cbs[q].ap()[:],
                        in_offset=IndirectOffsetOnAxis(ap=idxs[:, q:q + 1], axis=0),
                        compute_op=(AX.bypass if q == 0 else AX.add))
                if q < NCB - 1:
                    for tix in grp:
                        # residT = xT - transpose(out_acc)
                        tp = ps_tp.tile([128, 2 * 128], F32, tag="tp")
                        for j in range(2):
                            nc.tensor.transpose(
                                tp[:, 128 * j:128 * (j + 1)],
                                outq_g[tix][:, 128 * j:128 * (j + 1)], identity[:])
                        resid_new = resid_pool.tile([128, 2 * 128], F32, tag="resid")
                        nc.vector.tensor_tensor(
                            resid_new[:], xt_g[tix][:], tp[:], op=AX.subtract)
                        resid_g[tix] = resid_new

            for tix in grp:
                b, hb = divmod(tix, NHB)
                nc.sync.dma_start(oq[b, hb], outq_g[tix][:])
                nc.sync.dma_start(oi[b, hb], idx_g[tix][:])
                nc.sync.dma_start(out_m.ap()[tix], m8_g[tix][:])
                nc.sync.dma_start(out_xsq.ap()[tix], xsq_g[tix][:])

    nc.compile()
    return nc


def _split16(a):
    """exact 2-way fp16 split: a ~= h + l (h,l fp16)"""
    h = a.astype(np.float16)
    l = (a.astype(np.float64) - h.astype(np.float64)).astype(np.float16)
    return h, l


def _prep_host(encoded_x, codebooks):
    x = np.ascontiguousarray(np.asarray(encoded_x), dtype=np.float32)
    cb = np.ascontiguousarray(np.asarray(codebooks), dtype=np.float32)
    cbt = np.ascontiguousarray(np.transpose(cb, (0, 2, 1)))       # [4, 256, 512]
    ch, cl = _split16(cbt)
    b64 = -0.5 * (cb.astype(np.float64) ** 2).sum(-1)             # [4, 512]
    b_h = b64.astype(np.float16)
    r1 = b64 - b_h.astype(np.float64)
    b_m = r1.astype(np.float16)
    r2 = r1 - b_m.astype(np.float64)
    b_l = r2.astype(np.float16)
    bias3 = np.ascontiguousarray(np.stack([b_h, b_m, b_l], axis=1))  # [4, 3, 512]
    in_maps = []
    for i in range(NCORES):
        m = {"x": np.ascontiguousarray(x[i * BPC:(i + 1) * BPC]),
             "cbh": ch, "cbl": cl, "bias3": bias3}
        for q in range(NCB):
            m[f"cb{q}"] = cb[q]
        in_maps.append(m)
    return in_maps


def _assemble(results):
    qo = np.concatenate([results[i]["out_q"] for i in range(NCORES)], axis=0)
    idx = np.concatenate([results[i]["out_i"] for i in range(NCORES)],
                         axis=0).astype(np.int32)
    sum_xsq = sum(results[i]["out_xsq"].sum(dtype=np.float64) for i in range(NCORES))
    summax = [sum(results[i]["out_m"][:, :, 8 * q].sum(dtype=np.float64)
                  for i in range(NCORES)) for q in range(NCB)]
    n_el = float(B * H * W) * D
    losses, csum = [], 0.0
    for q in range(NCB):
        csum += summax[q]
        losses.append((sum_xsq - 2.0 * csum) / n_el)
    loss = np.array(losses, dtype=np.float32)
    return qo, idx, loss


_CACHED_NC = None


def kernel(encoded_x, codebooks):
    global _CACHED_NC
    if _CACHED_NC is None:
        _CACHED_NC = build_kernel()
    in_maps = _prep_host(encoded_x, codebooks)
    res = bass_utils.run_bass_kernel_spmd(_CACHED_NC, in_maps,
                                          core_ids=list(range(NCORES)))
    return _assemble(res.results)
